# revision 38
# baseline (speedup 1.0000x reference)
"""DGLJTNNDecoder forward on 8 Trainium2 NeuronCores (Bass/Tile).

v2: feature-major fp8(e4m3) DoubleRow rewrite of the fp16 baseline.

Strategy (data-parallel over trees, 128 trees/core, weights replicated):
  The reference's 46-step DFS scan is two independent 23-step GRU-style
  chains (forward / backward edges). All state is kept FEATURE-MAJOR
  ([feature-in-chunk, chunk, tree]) so the PSUM output of every gemm is
  already in the lhsT/rhs layout the next gemm needs -> zero transposes.

  Precision plan (validated in numpy against the exact inputs):
    - chain gemms (Wz2/Wh2/Ur), p-head (U2), q first layer (W1): fp8
      DoubleRow matmuls (0.5 cycles/col, K=256 per instruction).
      Weights host-scaled x16, gathered emb_pre terms x64 (added into
      PSUM via a 0.25*identity-pair DoubleRow matmul), descale by 1/16
      in the ACT activation that reads the PSUM.
    - m state: stored fp8 directly (the DVE update writes fp8; keeps the
      recurrence critical path short); rm produced fp8 directly (DVE STT).
    - q output layer (Wo) and tvW stay fp16: q_acc counts exact argmax
      matches (~30 of 24576) and fp8 logits would flip them.
  The p-logit dot (relu(.)*us summed over features) is a PE matmul with
  a one-hot-column us stationary, accumulating all 47 p-blocks into one
  PSUM bank [block, tree] -> the 47 per-block DVE reductions of the
  baseline disappear.

  Per-core output: [qloss_sum, ploss_sum, qcnt, pcnt_delta] fp32; host
  combines across cores into the reference's 4-scalar tuple.
"""

import numpy as np
import ml_dtypes
from contextlib import ExitStack

import concourse.bass as bass
import concourse.bacc as bacc
import concourse.mybir as mybir
import concourse.tile as tile
from concourse.bass_utils import run_bass_kernel_spmd

F8 = mybir.dt.float8e4
F16 = mybir.dt.float16
F32 = mybir.dt.float32
AF = mybir.ActivationFunctionType
ALU = mybir.AluOpType
AX = mybir.AxisListType
DR = mybir.MatmulPerfMode.DoubleRow
E4 = ml_dtypes.float8_e4m3

N_CORES = 8
T, L, H, LAT, V = 1024, 24, 450, 56, 780
TC = T // N_CORES          # 128 trees per core
NF = L - 1                 # 23 steps per chain
NE = 2 * NF

# step schedule: chain f step k: src=k dst=k+1 ; chain b step k: src=23-k
# dst=22-k. p block j: j=0 root(node 0), j=t+1 uses hs[t] and gp[dst[t]].
SRC_F = list(range(NF));        DST_F = [k + 1 for k in range(NF)]
SRC_B = [NF - k for k in range(NF)]; DST_B = [NF - 1 - k for k in range(NF)]

# DMA / storage order of nodes (chain-consumption order)
NODE_ORDER = []
for k in range(NF + 1):
    for n in (k, NF - k):
        if n not in NODE_ORDER:
            NODE_ORDER.append(n)
POS = {n: i for i, n in enumerate(NODE_ORDER)}


DEBUG_TAPS = False

def build_program():
    nc = bacc.Bacc("TRN2", target_bir_lowering=False, debug=False,
                   num_devices=N_CORES)

    din = {}
    def dram_in(name, shape, dtype):
        din[name] = nc.dram_tensor(name, list(shape), dtype,
                                   kind="ExternalInput").ap()
        return din[name]

    # per step-pair k: [p, kind(z|h|r), cout, tree*2] with cols 0:128 =
    # chain f (z/h: node src_f, r: node dst_f), 128:256 = chain b
    dram_in("gzp", [NF, 128, 3, 4, 256], F8)
    dram_in("gp", [L, 128, 4, 128], F8)        # node(POS) -> [p,c,tree]
    dram_in("qmask", [L, 128, V], F16)         # one-hot of wid targets
    for w in ("wz2", "wh2", "ur", "u2", "w1"):
        dram_in(w, [128, 4, 4, 128], F8)       # [fi, cin, cout, fo] x16
    dram_in("wo", [128, 4, V], F16)            # [fi, cin, v]; [66,3,:]=Wo_b
    dram_in("tvw", [128, 4, 128], F16)         # feature-major tvW x16
    dram_in("ust", [128, 4, L, 32], F16)       # us chunk c at one-hot col j
    dram_in("identp", [128, 3, 128], F8)       # [0 | 0.25*I | 0]
    dram_in("ident16", [128, 128], F16)
    dram_in("pw", [24, 256], F16)              # +-1 pcnt weights
    dram_in("scp", [24, 1], F32)               # exp scale (-1 fwd tgt1 else +1)
    dram_in("ones32", [128, 1], F32)
    out_d = nc.dram_tensor("out", [4, 1], F32, kind="ExternalOutput").ap()
    dbg = None
    if DEBUG_TAPS:
        dbg = {
            "d_psp": nc.dram_tensor("d_psp", [32, 256], F32,
                                    kind="ExternalOutput").ap(),
            "d_seq": nc.dram_tensor("d_seq", [128, L], F32,
                                    kind="ExternalOutput").ap(),
            "d_ltgt": nc.dram_tensor("d_ltgt", [128, L], F32,
                                     kind="ExternalOutput").ap(),
            "d_mx": nc.dram_tensor("d_mx", [128, L], F32,
                                   kind="ExternalOutput").ap(),
            "d_m8f": nc.dram_tensor("d_m8f", [NF, 128, 512], F8,
                                    kind="ExternalOutput").ap(),
            "d_m8b": nc.dram_tensor("d_m8b", [NF, 128, 512], F8,
                                    kind="ExternalOutput").ap(),
        }

    with tile.TileContext(nc) as tc, ExitStack() as ctx:
        _kern(ctx, tc, din, out_d, dbg)

    nc.compile()
    return nc


def _kern(ctx, tc, din, out_d, dbg=None):
    nc = tc.nc

    pc = ctx.enter_context(tc.tile_pool(name="const", bufs=1))
    pm = ctx.enter_context(tc.tile_pool(name="m", bufs=1))
    pacc = ctx.enter_context(tc.tile_pool(name="acc", bufs=1))
    # p-collect psum lives across both phases: [block-row, tree*2]
    ppc = ctx.enter_context(tc.tile_pool(name="pcol", bufs=1, space="PSUM"))

    def const_tile(name, shape, dtype):
        t = pc.tile(list(shape), dtype, tag=name, name=name)
        nc.sync.dma_start(t[:], din[name][:])
        return t

    # ---- input tiles; DMA issued in chain-consumption priority ----
    gz_t = pm.tile([128, NF, 3, 4, 256], F8, tag="gz", name="gz_t")
    gp_t = pm.tile([128, L, 4, 128], F8, tag="gpt", name="gp_t")
    src_gz = din["gzp"].rearrange("o p k c t -> p o k c t")
    src_gp = din["gp"].rearrange("o p c t -> p o c t")

    identp = const_tile("identp", [128, 3, 128], F8)
    nc.sync.dma_start(gz_t[:, 0:2], src_gz[:, 0:2])      # pairs 0,1
    nc.sync.dma_start(gp_t[:, 0:4], src_gp[:, 0:4])
    ur = const_tile("ur", [128, 4, 4, 128], F8)
    wz2 = const_tile("wz2", [128, 4, 4, 128], F8)
    wh2 = const_tile("wh2", [128, 4, 4, 128], F8)
    u2 = const_tile("u2", [128, 4, 4, 128], F8)
    ust = const_tile("ust", [128, 4, L, 32], F16)
    for i0, i1 in ((2, 6), (6, 11), (11, 17), (17, 23)):
        nc.sync.dma_start(gz_t[:, i0:i1], src_gz[:, i0:i1])
    for i in range(1, 6):
        nc.sync.dma_start(gp_t[:, 4 * i:4 * (i + 1)], src_gp[:, 4 * i:4 * (i + 1)])

    def gzk(k, kind):
        """[128, 2, 1024] view for pair k: bank blocks (c01|c23) x 256."""
        return gz_t[:, k, kind].rearrange("p (b c2) t -> p b (c2 t)", b=2)

    def gp2(n):
        """([128, 2, 512] node-pair view, second) selecting node n's gp."""
        p = POS[n]
        if p < L - 1:
            return gp_t[:, p:p + 2].rearrange("p o c t -> p o (c t)"), False
        return gp_t[:, p - 1:p + 1].rearrange("p o c t -> p o (c t)"), True

    ident16 = const_tile("ident16", [128, 128], F16)
    tvw = const_tile("tvw", [128, 4, 128], F16)
    w1 = const_tile("w1", [128, 4, 4, 128], F8)
    wo = const_tile("wo", [128, 4, V], F16)
    pw = const_tile("pw", [24, 256], F16)
    scp = const_tile("scp", [24, 1], F32)
    ones32 = const_tile("ones32", [128, 1], F32)

    # persistent state: fp8 m pair tiles [p, cin, tree*2] (f | b halves),
    # written directly by the DVE update (fp8 state keeps q_acc exact)
    m8 = [pm.tile([128, 4, 256], F8, tag=f"m8{k}", name=f"m8{k}")
          for k in range(NF)]
    m8f = [t[:, :, 0:128] for t in m8]      # chain-f halves
    m8b = [t[:, :, 128:256] for t in m8]
    hs8b = [pm.tile([128, 4, 128], F8, tag=f"hs8b{k}", name=f"hs8b{k}")
            for k in range(NF - 1)]   # k=22 uses m8b[22] directly

    # accumulation buffers
    seq_buf = pacc.tile([128, L], F32, tag="seq")
    ltgt_buf = pacc.tile([128, L], F32, tag="ltgt")
    mx_buf = pacc.tile([128, L], F32, tag="mx")
    partq = pacc.tile([128, 4], F32, tag="partq")
    partp = pacc.tile([24, 4], F32, tag="partp")
    nc.vector.memset(partq[:], 0.0)
    nc.vector.memset(partp[:], 0.0)

    # p-collect psum: partition j = p-block row (0=root, k+1=pair k),
    # cols 0:128 fwd-block logits, 128:256 bwd-block logits
    psp = ppc.tile([32, 256], F32, tag="pcol")

    def mm(out, lhsT, rhs, start, stop, pm_=None):
        nc.tensor.matmul(out, lhsT, rhs, start=start, stop=stop, perf_mode=pm_)

    def ga_add(ps, rhs2, second, stop=False):
        """psum[:, 0:512] = 0.25 * ga(x64), ONE DoubleRow instr: rhs2 is a
        [128, 2, 512] view of two adjacent 512-col blocks; the ident pair
        selects block 0 (second=False) or block 1 (second=True).
        PSUM start/stop semantics are BANK-granular (2KB zero region):
        this is the bank's single start instr."""
        lhsT = identp[:, 0:2, :] if second else identp[:, 1:3, :]
        mm(ps[:, 0:512], lhsT, rhs2, True, stop, DR)

    def gemm8(psv, w, x8):
        """psum[:, c, :] += sum_cin w[:,cin,c,:].T @ x8[:,cin,:] (x16 scale);
        closes the bank's accumulation group on the last instr."""
        for c in range(4):
            for cp in range(2):
                mm(psv[:, c, :], w[:, 2 * cp:2 * cp + 2, c, :],
                   x8[:, 2 * cp:2 * cp + 2, :],
                   False, c == 3 and cp == 1, DR)

    # ================= phase A: chains + root/forward p blocks ============
    with tc.tile_pool(name="Aps", bufs=1, space="PSUM") as pA, \
         tc.tile_pool(name="Asb", bufs=2) as sA:

        def p_block_us(pa, col, half, start, stop):
            """accumulate p logits: psp[col, half] += us . pa"""
            for c in range(4):
                mm(psp[:, 128 * half:128 * half + 128], ust[:, c, col, :],
                   pa[:, c, :], start and c == 0, stop and c == 3)

        def fwd_p_block(k):
            """p block j=k+1 (root when k=-1): relu(gp[dst] + hs@U2) . us"""
            node = 0 if k < 0 else DST_F[k]
            ps_p = pA.tile([128, 512], F32, tag="pf", bufs=2,
                           name=f"psp{k}")
            psv = ps_p[:].rearrange("p (c t) -> p c t", c=4)
            rhs2, sec = gp2(node)
            ga_add(ps_p, rhs2, sec, stop=(k < 0))
            if k >= 0:
                gemm8(psv, u2, m8f[k])
            pa = sA.tile([128, 4, 128], F16, tag="pa", name=f"pa{k}")
            nc.scalar.activation(pa[:].rearrange("p c t -> p (c t)"),
                                 ps_p[:], AF.Relu, scale=1.0 / 16.0)
            p_block_us(pa, k + 1, 0, k < 0, k == NF - 1)

        def ga_add2(ps, k, kind, stop=False):
            """both-chain ga into a [128, 1024] (2-bank) psum: one DR instr
            per bank, each the bank's single start."""
            v = gzk(k, kind)
            mm(ps[:, 0:512], identp[:, 1:3, :], v, True, stop, DR)
            mm(ps[:, 512:1024], identp[:, 0:2, :], v, True, stop, DR)

        def gemm8p(ps, w, x8):
            """pair gemm: psum [128, 4, 256] += W.T @ x8(pair); per-bank
            stop on the last instr of each bank (banks = couts 01 / 23)."""
            psv = ps[:].rearrange("p (c t) -> p c t", c=4, t=256)
            for c in range(4):
                for cp in range(2):
                    mm(psv[:, c, :], w[:, 2 * cp:2 * cp + 2, c, :],
                       x8[:, 2 * cp:2 * cp + 2, :],
                       False, c % 2 == 1 and cp == 1, DR)

        def chain_pair(k):
            ps_z = pA.tile([128, 1024], F32, tag="z", name=f"psz{k}")
            ga_add2(ps_z, k, 0, stop=(k == 0))
            if k > 0:
                gemm8p(ps_z, wz2, m8[k - 1])
            ps_h = pA.tile([128, 1024], F32, tag="h", name=f"psh{k}")
            ga_add2(ps_h, k, 1, stop=(k == 0))
            if k > 0:
                gemm8p(ps_h, wh2, rm8_prev[0])

            z16 = sA.tile([128, 4, 256], F16, tag="zt", name=f"zt{k}")
            nc.scalar.activation(z16[:].rearrange("p c t -> p (c t)"),
                                 ps_z[:], AF.Sigmoid, scale=1.0 / 16.0)
            mt16 = sA.tile([128, 4, 256], F16, tag="mt", name=f"mt{k}")
            nc.scalar.activation(mt16[:].rearrange("p c t -> p (c t)"),
                                 ps_h[:], AF.Tanh, scale=1.0 / 16.0)

            mk = m8[k]
            if k == 0:
                nc.vector.tensor_mul(mk[:], z16[:], mt16[:])
            else:
                s8 = m8[k - 1]
                d1 = sA.tile([128, 4, 256], F16, tag="d1", name=f"d1{k}")
                nc.vector.tensor_sub(d1[:], mt16[:], s8[:])
                d2 = sA.tile([128, 4, 256], F16, tag="d2", name=f"d2{k}")
                nc.vector.tensor_mul(d2[:], z16[:], d1[:])
                nc.vector.tensor_add(mk[:], s8[:], d2[:])

            if k == NF - 1:
                return
            ps_r = pA.tile([128, 1024], F32, tag="z", name=f"psr{k}")
            ga_add2(ps_r, k, 2)
            gemm8p(ps_r, ur, m8[k])
            r16 = sA.tile([128, 4, 256], F16, tag="rt", name=f"rt{k}")
            nc.scalar.activation(r16[:].rearrange("p c t -> p (c t)"),
                                 ps_r[:], AF.Sigmoid, scale=1.0 / 16.0)
            rmn = sA.tile([128, 4, 256], F8, tag="rm", name=f"rm{k}")
            nc.vector.scalar_tensor_tensor(rmn[:], r16[:], 1.0, mk[:],
                                           op0=ALU.mult, op1=ALU.mult)
            rm8_prev[0] = rmn

        rm8_prev = {}
        fwd_p_block(-1)                      # root
        for k in range(NF):
            chain_pair(k)
            fwd_p_block(k)

        # backward hs (fp8) on gpsimd: hs_b[k] = m_b[k] + m_f[dst-1]
        for k in range(NF - 1):
            nc.gpsimd.tensor_add(hs8b[k][:], m8b[k],
                                 m8f[DST_B[k] - 1])

    # ================= phase B: q blocks + backward p blocks ==============
    with tc.tile_pool(name="Bps", bufs=1, space="PSUM") as pB, \
         tc.tile_pool(name="Bsb", bufs=2) as sB, \
         tc.tile_pool(name="msk", bufs=1) as pmsk:

        src_qm = din["qmask"].rearrange("o p f -> p o f")
        qm_b = [pmsk.tile([128, V], F16, tag=f"qm{i}", name=f"qm{i}")
                for i in range(3)]
        for jj in range(3):
            nc.sync.dma_start(qm_b[jj][:], src_qm[:, jj, :])

        def p_block_us(pa, col, half, start, stop):
            for c in range(4):
                mm(psp[:, 128 * half:128 * half + 128], ust[:, c, col, :],
                   pa[:, c, :], start and c == 0, stop and c == 3)

        def bwd_p_block(k):
            """p block for backward step tb=NF+k: gp[dst_b] + hs_b[k]@U2"""
            node = DST_B[k]
            ps_p = pB.tile([128, 512], F32, tag="pb", bufs=1, name=f"pspb{k}")
            psv = ps_p[:].rearrange("p (c t) -> p c t", c=4)
            rhs2, sec = gp2(node)
            ga_add(ps_p, rhs2, sec)
            hsrc = m8b[k] if k == NF - 1 else hs8b[k][:]
            gemm8(psv, u2, hsrc)
            pa = sB.tile([128, 4, 128], F16, tag="pab", name=f"pab{k}")
            nc.scalar.activation(pa[:].rearrange("p c t -> p (c t)"),
                                 ps_p[:], AF.Relu, scale=1.0 / 16.0)
            p_block_us(pa, k + 1, 1, k == 0, k == NF - 1)

        for j in range(L):
            if j < NF:
                bwd_p_block(j)

            # q block j: act = relu(tvW + hs[j-1] @ W1), feature-major
            ps_qa = pB.tile([128, 512], F32, tag="qa", bufs=1, name=f"psqa{j}")
            qv = ps_qa[:].rearrange("p (c t) -> p c t", c=4)
            mm(ps_qa[:, 0:512], ident16[:, :],
               tvw[:].rearrange("p c t -> p (c t)"), True, j == 0)
            if j > 0:
                gemm8(qv, w1, m8f[j - 1])
            qa = sB.tile([128, 4, 128], F16, tag="qat", name=f"qa{j}")
            # host sets tvw[66,3,:]=16 -> relu(16/16)=1.0 = the Wo_b ones-row
            nc.scalar.activation(qa[:].rearrange("p c t -> p (c t)"),
                                 ps_qa[:], AF.Relu, scale=1.0 / 16.0)

            ps_log = pB.tile([128, 1024], F32, tag="log", bufs=2,
                             name=f"pslog{j}")
            KCQ = [128, 128, 128, 67]
            for v0, v1 in ((0, 512), (512, V)):
                for c in range(4):
                    mm(ps_log[:, v0:v1], qa[0:KCQ[c], c, :],
                       wo[0:KCQ[c], c, v0:v1], c == 0, c == 3)

            nc.vector.tensor_reduce(mx_buf[:, j:j + 1], ps_log[:, 0:V],
                                    axis=AX.X, op=ALU.max)
            exp_s = sB.tile([128, V], F16, tag="exps", name=f"exps{j}")
            nc.scalar.activation(exp_s[:], ps_log[:, 0:V], AF.Exp,
                                 accum_out=seq_buf[:, j:j + 1])
            ttr_s = sB.tile([128, V], F16, tag="ttrs", name=f"ttrs{j}")
            nc.vector.scalar_tensor_tensor(
                ttr_s[:], ps_log[:, 0:V], 1.0, qm_b[j % 3][:],
                op0=ALU.mult, op1=ALU.mult,
                accum_out=ltgt_buf[:, j:j + 1])
            if j + 3 < L:
                nc.sync.dma_start(qm_b[j % 3][:], src_qm[:, j + 3, :])

        if dbg is not None:
            pspc = sB.tile([32, 256], F32, tag="dbgpsp")
            nc.scalar.copy(pspc[:], psp[:])
            nc.sync.dma_start(dbg["d_psp"][:], pspc[:])
            nc.sync.dma_start(dbg["d_seq"][:], seq_buf[:])
            nc.sync.dma_start(dbg["d_ltgt"][:], ltgt_buf[:])
            nc.sync.dma_start(dbg["d_mx"][:], mx_buf[:])
            for k in range(NF):
                nc.sync.dma_start(
                    dbg["d_m8f"][k],
                    m8f[k].rearrange("p c t -> p (c t)"))
                nc.sync.dma_start(
                    dbg["d_m8b"][k],
                    m8b[k].rearrange("p c t -> p (c t)"))

        # ---------------- tails ----------------
        # q: loss + argmax-match count
        lnseq = sB.tile([128, L], F32, tag="lnseq")
        nc.scalar.activation(lnseq[:], seq_buf[:], AF.Ln)
        qcol = sB.tile([128, L], F32, tag="qcol")
        nc.vector.tensor_sub(qcol[:], lnseq[:], ltgt_buf[:])
        nc.vector.tensor_reduce(partq[:, 0:1], qcol[:], axis=AX.X, op=ALU.add)
        qeq = sB.tile([128, L], F32, tag="qeq")
        nc.vector.tensor_tensor(qeq[:], ltgt_buf[:], mx_buf[:], ALU.is_equal)
        nc.vector.tensor_reduce(partq[:, 2:3], qeq[:], axis=AX.X, op=ALU.add)

        # p: read collected logits [24, 256]
        gtb = sB.tile([24, 256], F32, tag="gtb")
        nc.vector.tensor_scalar(gtb[:], psp[0:24, :], 0.0, None, op0=ALU.is_gt)
        pdj = sB.tile([24, 256], F32, tag="pdj")
        nc.vector.scalar_tensor_tensor(pdj[:], gtb[:], 1.0, pw[:],
                                       op0=ALU.mult, op1=ALU.mult,
                                       accum_out=partp[:, 3:4])
        # softplus: fwd half signed by scp (-1 for target-1 rows), bwd half +1
        e1 = sB.tile([24, 128], F32, tag="e1")
        nc.scalar.activation(e1[:], psp[0:24, 0:128], AF.Exp, scale=scp[:, 0:1])
        l1 = sB.tile([24, 128], F32, tag="l1")
        nc.scalar.activation(l1[:], e1[:], AF.Ln, bias=ones32[0:24, 0:1])
        nc.vector.tensor_reduce(partp[:, 1:2], l1[:], axis=AX.X, op=ALU.add)
        # row 0 of the bwd half is exact zeros (start-zeroed, never written):
        # contributes 128*softplus(0) = 128*ln2, subtracted on the host.
        e0 = sB.tile([24, 128], F32, tag="e0")
        nc.scalar.activation(e0[:], psp[0:24, 128:256], AF.Exp)
        l0 = sB.tile([24, 128], F32, tag="l0")
        nc.scalar.activation(l0[:], e0[:], AF.Ln, bias=ones32[0:24, 0:1])
        ps0 = sB.tile([24, 1], F32, tag="ps0")
        nc.vector.tensor_reduce(ps0[:], l0[:], axis=AX.X, op=ALU.add)
        nc.vector.tensor_add(partp[:, 1:2], partp[:, 1:2], ps0[:])

        # final partition reduction: out[4,1] = partq.T@1 + partp.T@1
        ps_out = pB.tile([4, 1], F32, tag="out", bufs=1)
        nc.tensor.matmul(ps_out[:], partq[:, :], ones32[:, :],
                         start=True, stop=False)
        nc.tensor.matmul(ps_out[:], partp[:, :], ones32[0:24, :],
                         start=False, stop=True)
        out_sb = sB.tile([4, 1], F32, tag="outsb")
        nc.scalar.copy(out_sb[:], ps_out[:])
        nc.sync.dma_start(out_d[:], out_sb[:])


# ------------------------------------------------------------------
_PROGRAM = None

def _get_program():
    global _PROGRAM
    if _PROGRAM is None:
        _PROGRAM = build_program()
    return _PROGRAM


def _to_e4(x, scale):
    return np.clip(np.asarray(x, np.float32) * scale, -224.0, 224.0).astype(E4)


def _pack_w8(w):
    """[450, 450] fp32 -> [128, 4, 4, 128] fp8 x16 (fi,cin,cout,fo)"""
    out = np.zeros((128, 4, 4, 128), E4)
    wp = np.zeros((512, 512), np.float32)
    wp[:450, :450] = w
    v = wp.reshape(4, 128, 4, 128)          # [cin, fi, cout, fo]
    out[:] = _to_e4(np.transpose(v, (1, 0, 2, 3)), 16.0)
    return out


def _fm16(x, scale=1.0):
    """[trees(128), 450] fp32 -> [128, 4, 128] fp16 feature-major"""
    out = np.zeros((128, 4, 128), np.float16)
    xp = np.zeros((x.shape[0], 512), np.float32)
    xp[:, :450] = x * scale
    out[:] = np.transpose(xp.reshape(-1, 4, 128), (2, 1, 0))
    return out


def make_in_maps(wid, tree_vec, emb, W_w, W_b, U_w, U_b, Wo_w, Wo_b, Us_w,
                 Us_b, Wz_w, Wz_b, Wr_w, Ur_w, Ur_b, Wh_w, Wh_b):
    """Host-side shard + pack. Returns list of 8 per-core input dicts."""
    f32 = np.float32
    wid = np.asarray(wid); emb = np.asarray(emb, f32)
    tree_vec = np.asarray(tree_vec, f32)
    W_w, W_b = np.asarray(W_w, f32), np.asarray(W_b, f32)
    U_w, U_b = np.asarray(U_w, f32), np.asarray(U_b, f32)
    Wz_w, Wz_b = np.asarray(Wz_w, f32), np.asarray(Wz_b, f32)
    Wh_w, Wh_b = np.asarray(Wh_w, f32), np.asarray(Wh_b, f32)
    Wr_w = np.asarray(Wr_w, f32)
    Ur_w, Ur_b = np.asarray(Ur_w, f32), np.asarray(Ur_b, f32)
    Wo_w, Wo_b = np.asarray(Wo_w, f32), np.asarray(Wo_b, f32)
    Us_w, Us_b = np.asarray(Us_w, f32), np.asarray(Us_b, f32)
    assert float(Us_b[0]) == 0.0

    # vocab-sized precompute: [emb,1] @ [Wz1|Wh1|Wr|U1] + bias rows
    wpre = np.concatenate([Wz_w[:H], Wh_w[:H], Wr_w, U_w[:H]], 1)
    bias_row = np.concatenate([Wz_b, Wh_b, Ur_b, np.zeros(H, f32)])
    emb_pre = emb @ wpre + bias_row[None, :]           # [V, 1800]
    tvU = tree_vec @ U_w[2 * H:] + U_b                 # [T, H]
    tvW = tree_vec @ W_w[H:H + LAT] + W_b[None, :]     # [T, H]

    wo16 = np.zeros((128, 4, V), np.float16)
    wop = np.zeros((512, V), np.float32)
    wop[:450] = Wo_w
    wop[450] = Wo_b                                    # ones-row K slot
    wo16[:] = np.transpose(wop.reshape(4, 128, V), (1, 0, 2))

    us = Us_w[:, 0]
    usp = np.zeros(512, f32); usp[:450] = us
    ust = np.zeros((128, 4, L, 32), np.float16)
    for j in range(L):
        ust[:, :, j, j] = usp.reshape(4, 128).T

    identp = np.zeros((128, 3, 128), E4)
    identp[:, 1, :] = (np.eye(128) * 0.25).astype(E4)

    pwh = np.zeros((24, 256), np.float16)
    pwh[0, 0:128] = 1.0                                # root, target 1
    pwh[1:23, 0:128] = 1.0                             # fwd k=0..21, target 1
    pwh[23, 0:128] = -1.0                              # fwd k=22, target 0
    pwh[1:24, 128:256] = -1.0                          # bwd, target 0
    scph = np.full((24, 1), -1.0, f32)
    scph[23, 0] = 1.0

    shared = {
        "wz2": _pack_w8(Wz_w[H:]), "wh2": _pack_w8(Wh_w[H:]),
        "ur": _pack_w8(Ur_w), "u2": _pack_w8(U_w[H:2 * H]),
        "w1": _pack_w8(W_w[:H]),
        "wo": wo16, "ust": ust, "identp": identp,
        "ident16": np.eye(128, dtype=np.float16),
        "pw": pwh, "scp": scph,
        "ones32": np.ones((128, 1), f32),
    }

    in_maps = []
    for core in range(N_CORES):
        t0 = core * TC
        wc = wid[t0:t0 + TC]                           # [128, L]
        g = emb_pre[wc]                                # [128, L, 1800]
        # per-node feature-major fp8 views of the three chain kinds
        gfm = np.zeros((L, 128, 3, 4, 128), np.float32)
        gpn = np.zeros((L, 128, 4, 128), E4)
        for n in range(L):
            gn = g[:, n]                               # [tree, 1800]
            gpad = np.zeros((TC, 3, 512), np.float32)
            for kind in range(3):
                gpad[:, kind, :450] = gn[:, kind * H:(kind + 1) * H]
            v = gpad.reshape(TC, 3, 4, 128)
            gfm[n] = np.transpose(v, (3, 1, 2, 0))     # [p, kind, c, tree]
            gpp = np.zeros((TC, 512), np.float32)
            gpp[:, :450] = gn[:, 3 * H:] + tvU[t0:t0 + TC]
            gpn[POS[n]] = _to_e4(
                np.transpose(gpp.reshape(TC, 4, 128), (2, 1, 0)), 64.0)
        # step-pair packing: [pair, p, kind, c, 256] (f | b halves)
        gzp = np.zeros((NF, 128, 3, 4, 256), E4)
        for k in range(NF):
            for half, (nzh, nr) in enumerate(
                    ((SRC_F[k], DST_F[k]), (SRC_B[k], DST_B[k]))):
                blk = np.empty((128, 3, 4, 128), np.float32)
                blk[:, 0:2] = gfm[nzh][:, 0:2]
                blk[:, 2] = gfm[nr][:, 2]
                gzp[k, :, :, :, 128 * half:128 * (half + 1)] = \
                    _to_e4(blk, 64.0)

        qm = np.zeros((L, TC, V), np.float16)
        jj, tt = np.meshgrid(np.arange(L), np.arange(TC), indexing="ij")
        qm[jj, tt, wc.T] = 1.0

        m = dict(shared)
        m["gzp"] = gzp
        m["gp"] = gpn
        m["qmask"] = qm
        tvw16 = _fm16(tvW[t0:t0 + TC], 16.0)
        tvw16[66, 3, :] = 16.0      # -> relu(16/16)=1.0: Wo_b ones-row
        m["tvw"] = tvw16
        in_maps.append(m)
    return in_maps


def combine(outs):
    """outs: list of 8 [4,1] arrays -> reference 4-tuple."""
    s = np.sum([o[:, 0].astype(np.float64) for o in outs], axis=0)
    q_loss = np.float32(s[0] / T)
    p_loss = np.float32((s[1] - N_CORES * TC * np.log(2.0)) / T)
    q_acc = np.float32(np.float32(s[2]) / np.float32(L * T))
    p_cnt = s[3] + N_CORES * 24 * TC
    p_acc = np.float32(np.float32(p_cnt) / np.float32((NE + 1) * T))
    return (q_loss, p_loss, q_acc, p_acc)


def run_on_cores(in_maps, trace=False, **kw):
    nc = _get_program()
    return run_bass_kernel_spmd(nc, in_maps, list(range(N_CORES)),
                                trace=trace, **kw)


def kernel(**inputs):
    in_maps = make_in_maps(**inputs)
    res = run_on_cores(in_maps)
    return combine([res.results[c]["out"] for c in range(N_CORES)])


# revision 41
# speedup vs baseline: 1.2782x; 1.2782x over previous
"""DGLJTNNDecoder forward on 8 Trainium2 NeuronCores (Bass/Tile).

v2: feature-major fp8(e4m3) DoubleRow rewrite of the fp16 baseline.

Strategy (data-parallel over trees, 128 trees/core, weights replicated):
  The reference's 46-step DFS scan is two independent 23-step GRU-style
  chains (forward / backward edges). All state is kept FEATURE-MAJOR
  ([feature-in-chunk, chunk, tree]) so the PSUM output of every gemm is
  already in the lhsT/rhs layout the next gemm needs -> zero transposes.

  Precision plan (validated in numpy against the exact inputs):
    - chain gemms (Wz2/Wh2/Ur), p-head (U2), q first layer (W1): fp8
      DoubleRow matmuls (0.5 cycles/col, K=256 per instruction).
      Weights host-scaled x16, gathered emb_pre terms x64 (added into
      PSUM via a 0.25*identity-pair DoubleRow matmul), descale by 1/16
      in the ACT activation that reads the PSUM.
    - m state: stored fp8 directly (the DVE update writes fp8; keeps the
      recurrence critical path short); rm produced fp8 directly (DVE STT).
    - q output layer (Wo) and tvW stay fp16: q_acc counts exact argmax
      matches (~30 of 24576) and fp8 logits would flip them.
  The p-logit dot (relu(.)*us summed over features) is a PE matmul with
  a one-hot-column us stationary, accumulating all 47 p-blocks into one
  PSUM bank [block, tree] -> the 47 per-block DVE reductions of the
  baseline disappear.

  Per-core output: [qloss_sum, ploss_sum, qcnt, pcnt_delta] fp32; host
  combines across cores into the reference's 4-scalar tuple.
"""

import numpy as np
import ml_dtypes
from contextlib import ExitStack

import concourse.bass as bass
import concourse.bacc as bacc
import concourse.mybir as mybir
import concourse.tile as tile
from concourse.bass_utils import run_bass_kernel_spmd

F8 = mybir.dt.float8e4
F16 = mybir.dt.float16
F32 = mybir.dt.float32
AF = mybir.ActivationFunctionType
ALU = mybir.AluOpType
AX = mybir.AxisListType
DR = mybir.MatmulPerfMode.DoubleRow
E4 = ml_dtypes.float8_e4m3

N_CORES = 8
T, L, H, LAT, V = 1024, 24, 450, 56, 780
TC = T // N_CORES          # 128 trees per core
NF = L - 1                 # 23 steps per chain
NE = 2 * NF

# step schedule: chain f step k: src=k dst=k+1 ; chain b step k: src=23-k
# dst=22-k. p block j: j=0 root(node 0), j=t+1 uses hs[t] and gp[dst[t]].
SRC_F = list(range(NF));        DST_F = [k + 1 for k in range(NF)]
SRC_B = [NF - k for k in range(NF)]; DST_B = [NF - 1 - k for k in range(NF)]

# DMA / storage order of nodes (chain-consumption order)
NODE_ORDER = []
for k in range(NF + 1):
    for n in (k, NF - k):
        if n not in NODE_ORDER:
            NODE_ORDER.append(n)
POS = {n: i for i, n in enumerate(NODE_ORDER)}


DEBUG_TAPS = False

def build_program():
    nc = bacc.Bacc("TRN2", target_bir_lowering=False, debug=False,
                   num_devices=N_CORES)

    din = {}
    def dram_in(name, shape, dtype):
        din[name] = nc.dram_tensor(name, list(shape), dtype,
                                   kind="ExternalInput").ap()
        return din[name]

    dram_in("gzhr", [L, 128, 3, 4, 128], F8)   # node(POS) -> [p,kind,c,tree]
    dram_in("gp", [L, 128, 4, 128], F8)        # node(POS) -> [p,c,tree]
    dram_in("qmask", [L, 128, V], F16)         # one-hot of wid targets
    for w in ("wz2", "wh2", "ur", "u2", "w1"):
        dram_in(w, [128, 4, 4, 128], F8)       # [fi, cin, cout, fo] x16
    dram_in("wo", [128, 4, V], F16)            # [fi, cin, v]; [66,3,:]=Wo_b
    dram_in("tvw", [128, 4, 128], F16)         # feature-major tvW x16
    dram_in("ust", [128, 4, L, 32], F16)       # us chunk c at one-hot col j
    dram_in("identp", [128, 3, 128], F8)       # [0 | 0.25*I | 0]
    dram_in("ident16", [128, 128], F16)
    dram_in("pw", [24, 256], F16)              # +-1 pcnt weights
    dram_in("scp", [24, 1], F32)               # exp scale (-1 fwd tgt1 else +1)
    dram_in("ones32", [128, 1], F32)
    out_d = nc.dram_tensor("out", [4, 1], F32, kind="ExternalOutput").ap()
    dbg = None
    if DEBUG_TAPS:
        dbg = {
            "d_psp": nc.dram_tensor("d_psp", [32, 256], F32,
                                    kind="ExternalOutput").ap(),
            "d_seq": nc.dram_tensor("d_seq", [128, L], F32,
                                    kind="ExternalOutput").ap(),
            "d_ltgt": nc.dram_tensor("d_ltgt", [128, L], F32,
                                     kind="ExternalOutput").ap(),
            "d_mx": nc.dram_tensor("d_mx", [128, L], F32,
                                   kind="ExternalOutput").ap(),
            "d_m8f": nc.dram_tensor("d_m8f", [NF, 128, 512], F8,
                                    kind="ExternalOutput").ap(),
            "d_m8b": nc.dram_tensor("d_m8b", [NF, 128, 512], F8,
                                    kind="ExternalOutput").ap(),
        }

    with tile.TileContext(nc) as tc, ExitStack() as ctx:
        _kern(ctx, tc, din, out_d, dbg)

    nc.compile()
    return nc


def _kern(ctx, tc, din, out_d, dbg=None):
    nc = tc.nc

    pc = ctx.enter_context(tc.tile_pool(name="const", bufs=1))
    pm = ctx.enter_context(tc.tile_pool(name="m", bufs=1))
    pacc = ctx.enter_context(tc.tile_pool(name="acc", bufs=1))
    # p-collect psum lives across both phases: [block-row, tree*2]
    ppc = ctx.enter_context(tc.tile_pool(name="pcol", bufs=1, space="PSUM"))

    def const_tile(name, shape, dtype):
        t = pc.tile(list(shape), dtype, tag=name, name=name)
        nc.sync.dma_start(t[:], din[name][:])
        return t

    # ---- input tiles; DMA issued in chain-consumption priority ----
    gz_t = pm.tile([128, L, 3, 4, 128], F8, tag="gz", name="gz_t")
    gp_t = pm.tile([128, L, 4, 128], F8, tag="gpt", name="gp_t")
    src_gz = din["gzhr"].rearrange("o p k c t -> p o k c t")
    src_gp = din["gp"].rearrange("o p c t -> p o c t")

    identp = const_tile("identp", [128, 3, 128], F8)
    nc.sync.dma_start(gz_t[:, 0:4], src_gz[:, 0:4])      # nodes 0,23,1,22
    nc.sync.dma_start(gp_t[:, 0:4], src_gp[:, 0:4])
    ur = const_tile("ur", [128, 4, 4, 128], F8)
    wz2 = const_tile("wz2", [128, 4, 4, 128], F8)
    wh2 = const_tile("wh2", [128, 4, 4, 128], F8)
    u2 = const_tile("u2", [128, 4, 4, 128], F8)
    ust = const_tile("ust", [128, 4, L, 32], F16)
    for i in range(1, 6):
        nc.sync.dma_start(gz_t[:, 4 * i:4 * (i + 1)], src_gz[:, 4 * i:4 * (i + 1)])
        nc.sync.dma_start(gp_t[:, 4 * i:4 * (i + 1)], src_gp[:, 4 * i:4 * (i + 1)])
    def gz2(n, k0):
        """[128, 2, 512] view of kinds (k0, k0+1) for node n."""
        return gz_t[:, POS[n], k0:k0 + 2].rearrange("p k c t -> p k (c t)")

    def gp2(n):
        """([128, 2, 512] node-pair view, second) selecting node n's gp."""
        p = POS[n]
        if p < L - 1:
            return gp_t[:, p:p + 2].rearrange("p o c t -> p o (c t)"), False
        return gp_t[:, p - 1:p + 1].rearrange("p o c t -> p o (c t)"), True

    ident16 = const_tile("ident16", [128, 128], F16)
    tvw = const_tile("tvw", [128, 4, 128], F16)
    w1 = const_tile("w1", [128, 4, 4, 128], F8)
    wo = const_tile("wo", [128, 4, V], F16)
    pw = const_tile("pw", [24, 256], F16)
    scp = const_tile("scp", [24, 1], F32)
    ones32 = const_tile("ones32", [128, 1], F32)

    # persistent state: fp8 m (written directly by the DVE update; the
    # numpy validation run shows fp8 state keeps q_acc exact)
    m8 = {}
    for ch in ("f", "b"):
        for k in range(NF):
            m8[ch, k] = pm.tile([128, 4, 128], F8, tag=f"m8{ch}{k}",
                                name=f"m8{ch}{k}")
    hs8b = [pm.tile([128, 4, 128], F8, tag=f"hs8b{k}", name=f"hs8b{k}")
            for k in range(NF - 1)]   # k=22 uses m8[b,22] directly

    # accumulation buffers
    seq_buf = pacc.tile([128, L], F32, tag="seq")
    ltgt_buf = pacc.tile([128, L], F32, tag="ltgt")
    mx_buf = pacc.tile([128, L], F32, tag="mx")
    partq = pacc.tile([128, 4], F32, tag="partq")
    partp = pacc.tile([24, 4], F32, tag="partp")
    nc.vector.memset(partq[:], 0.0)
    nc.vector.memset(partp[:], 0.0)

    # p-collect psum: partition j = p-block row (0=root, k+1=pair k),
    # cols 0:128 fwd-block logits, 128:256 bwd-block logits
    psp = ppc.tile([32, 256], F32, tag="pcol")

    def mm(out, lhsT, rhs, start, stop, pm_=None):
        nc.tensor.matmul(out, lhsT, rhs, start=start, stop=stop, perf_mode=pm_)

    def ga_add(ps, rhs2, second, stop=False):
        """psum[:, 0:512] = 0.25 * ga(x64), ONE DoubleRow instr: rhs2 is a
        [128, 2, 512] view of two adjacent 512-col blocks; the ident pair
        selects block 0 (second=False) or block 1 (second=True).
        PSUM start/stop semantics are BANK-granular (2KB zero region):
        this is the bank's single start instr."""
        lhsT = identp[:, 0:2, :] if second else identp[:, 1:3, :]
        mm(ps[:, 0:512], lhsT, rhs2, True, stop, DR)

    def gemm8(psv, w, x8):
        """psum[:, c, :] += sum_cin w[:,cin,c,:].T @ x8[:,cin,:] (x16 scale);
        closes the bank's accumulation group on the last instr."""
        for c in range(4):
            for cp in range(2):
                mm(psv[:, c, :], w[:, 2 * cp:2 * cp + 2, c, :],
                   x8[:, 2 * cp:2 * cp + 2, :],
                   False, c == 3 and cp == 1, DR)

    # ================= phase A: chains + root/forward p blocks ============
    with tc.tile_pool(name="Aps", bufs=1, space="PSUM") as pA, \
         tc.tile_pool(name="Asb", bufs=2) as sA:

        def p_block_us(pa, col, half, start, stop):
            """accumulate p logits: psp[col, half] += us . pa"""
            for c in range(4):
                mm(psp[:, 128 * half:128 * half + 128], ust[:, c, col, :],
                   pa[:, c, :], start and c == 0, stop and c == 3)

        def fwd_p_block(k):
            """p block j=k+1 (root when k=-1): relu(gp[dst] + hs@U2) . us"""
            node = 0 if k < 0 else DST_F[k]
            ps_p = pA.tile([128, 512], F32, tag="pf", bufs=1,
                           name=f"psp{k}")
            psv = ps_p[:].rearrange("p (c t) -> p c t", c=4)
            rhs2, sec = gp2(node)
            ga_add(ps_p, rhs2, sec, stop=(k < 0))
            if k >= 0:
                gemm8(psv, u2, m8["f", k])
            pa = sA.tile([128, 4, 128], F16, tag="pa", name=f"pa{k}")
            nc.scalar.activation(pa[:].rearrange("p c t -> p (c t)"),
                                 ps_p[:], AF.Relu, scale=1.0 / 16.0)
            p_block_us(pa, k + 1, 0, k < 0, k == NF - 1)

        def chain_step(ch, k, srcs, dsts):
            src_n, dst_n = srcs[k], dsts[k]
            t_prev = (ch, k - 1)
            ps_z = pA.tile([128, 512], F32, tag=f"z{ch}", name=f"psz{ch}{k}")
            zv = ps_z[:].rearrange("p (c t) -> p c t", c=4)
            ga_add(ps_z, gz2(src_n, 0), False, stop=(k == 0))
            if k > 0:
                gemm8(zv, wz2, m8[t_prev])
            ps_h = pA.tile([128, 512], F32, tag=f"h{ch}", bufs=2,
                           name=f"psh{ch}{k}")
            hv = ps_h[:].rearrange("p (c t) -> p c t", c=4)
            ga_add(ps_h, gz2(src_n, 1), False, stop=(k == 0))
            if k > 0:
                gemm8(hv, wh2, rm8_prev[ch])

            z16 = sA.tile([128, 4, 128], F16, tag=f"zt{ch}", name=f"zt{ch}{k}")
            nc.scalar.activation(z16[:].rearrange("p c t -> p (c t)"),
                                 ps_z[:], AF.Sigmoid, scale=1.0 / 16.0)
            mt16 = sA.tile([128, 4, 128], F16, tag=f"mt{ch}", name=f"mt{ch}{k}")
            nc.scalar.activation(mt16[:].rearrange("p c t -> p (c t)"),
                                 ps_h[:], AF.Tanh, scale=1.0 / 16.0)

            mk = m8[ch, k]
            if k == 0:
                nc.vector.tensor_mul(mk[:], z16[:], mt16[:])
            else:
                s8 = m8[t_prev]
                d1 = sA.tile([128, 4, 128], F16, tag=f"d1{ch}", name=f"d1{ch}{k}")
                nc.vector.tensor_sub(d1[:], mt16[:], s8[:])
                d2 = sA.tile([128, 4, 128], F16, tag=f"d2{ch}", name=f"d2{ch}{k}")
                nc.vector.tensor_mul(d2[:], z16[:], d1[:])
                nc.vector.tensor_add(mk[:], s8[:], d2[:])

            if k == NF - 1:
                return
            ps_r = pA.tile([128, 512], F32, tag=f"z{ch}", name=f"psr{ch}{k}")
            rv = ps_r[:].rearrange("p (c t) -> p c t", c=4)
            ga_add(ps_r, gz2(dst_n, 1), True)
            gemm8(rv, ur, m8[ch, k])
            r16 = sA.tile([128, 4, 128], F16, tag=f"rt{ch}", name=f"rt{ch}{k}")
            nc.scalar.activation(r16[:].rearrange("p c t -> p (c t)"),
                                 ps_r[:], AF.Sigmoid, scale=1.0 / 16.0)
            rmn = sA.tile([128, 4, 128], F8, tag=f"rm{ch}", name=f"rm{ch}{k}")
            nc.vector.scalar_tensor_tensor(rmn[:], r16[:], 1.0, mk[:],
                                           op0=ALU.mult, op1=ALU.mult)
            rm8_prev[ch] = rmn

        rm8_prev = {}
        fwd_p_block(-1)                      # root
        for k in range(NF):
            chain_step("f", k, SRC_F, DST_F)
            chain_step("b", k, SRC_B, DST_B)
            fwd_p_block(k)

        # backward hs (fp8) on gpsimd: hs_b[k] = m_b[k] + m_f[dst-1]
        for k in range(NF - 1):
            nc.gpsimd.tensor_add(hs8b[k][:], m8["b", k][:],
                                 m8["f", DST_B[k] - 1][:])

    # ================= phase B: q blocks + backward p blocks ==============
    with tc.tile_pool(name="Bps", bufs=1, space="PSUM") as pB, \
         tc.tile_pool(name="Bsb", bufs=2) as sB, \
         tc.tile_pool(name="msk", bufs=1) as pmsk:

        src_qm = din["qmask"].rearrange("o p f -> p o f")
        qm_b = [pmsk.tile([128, V], F16, tag=f"qm{i}", name=f"qm{i}")
                for i in range(3)]
        for jj in range(3):
            nc.sync.dma_start(qm_b[jj][:], src_qm[:, jj, :])

        def p_block_us(pa, col, half, start, stop):
            for c in range(4):
                mm(psp[:, 128 * half:128 * half + 128], ust[:, c, col, :],
                   pa[:, c, :], start and c == 0, stop and c == 3)

        def bwd_p_block(k):
            """p block for backward step tb=NF+k: gp[dst_b] + hs_b[k]@U2"""
            node = DST_B[k]
            ps_p = pB.tile([128, 512], F32, tag="pb", bufs=1, name=f"pspb{k}")
            psv = ps_p[:].rearrange("p (c t) -> p c t", c=4)
            rhs2, sec = gp2(node)
            ga_add(ps_p, rhs2, sec)
            hsrc = m8["b", k] if k == NF - 1 else hs8b[k]
            gemm8(psv, u2, hsrc)
            pa = sB.tile([128, 4, 128], F16, tag="pab", name=f"pab{k}")
            nc.scalar.activation(pa[:].rearrange("p c t -> p (c t)"),
                                 ps_p[:], AF.Relu, scale=1.0 / 16.0)
            p_block_us(pa, k + 1, 1, k == 0, k == NF - 1)

        for j in range(L):
            if j < NF:
                bwd_p_block(j)

            # q block j: act = relu(tvW + hs[j-1] @ W1), feature-major
            ps_qa = pB.tile([128, 512], F32, tag="qa", bufs=1, name=f"psqa{j}")
            qv = ps_qa[:].rearrange("p (c t) -> p c t", c=4)
            mm(ps_qa[:, 0:512], ident16[:, :],
               tvw[:].rearrange("p c t -> p (c t)"), True, j == 0)
            if j > 0:
                gemm8(qv, w1, m8["f", j - 1])
            qa = sB.tile([128, 4, 128], F16, tag="qat", name=f"qa{j}")
            # host sets tvw[66,3,:]=16 -> relu(16/16)=1.0 = the Wo_b ones-row
            nc.scalar.activation(qa[:].rearrange("p c t -> p (c t)"),
                                 ps_qa[:], AF.Relu, scale=1.0 / 16.0)

            ps_log = pB.tile([128, 1024], F32, tag="log", bufs=2,
                             name=f"pslog{j}")
            KCQ = [128, 128, 128, 67]
            for v0, v1 in ((0, 512), (512, V)):
                for c in range(4):
                    mm(ps_log[:, v0:v1], qa[0:KCQ[c], c, :],
                       wo[0:KCQ[c], c, v0:v1], c == 0, c == 3)

            nc.vector.tensor_reduce(mx_buf[:, j:j + 1], ps_log[:, 0:V],
                                    axis=AX.X, op=ALU.max)
            exp_s = sB.tile([128, V], F16, tag="exps", name=f"exps{j}")
            nc.scalar.activation(exp_s[:], ps_log[:, 0:V], AF.Exp,
                                 accum_out=seq_buf[:, j:j + 1])
            ttr_s = sB.tile([128, V], F16, tag="ttrs", name=f"ttrs{j}")
            nc.vector.scalar_tensor_tensor(
                ttr_s[:], ps_log[:, 0:V], 1.0, qm_b[j % 3][:],
                op0=ALU.mult, op1=ALU.mult,
                accum_out=ltgt_buf[:, j:j + 1])
            if j + 3 < L:
                nc.sync.dma_start(qm_b[j % 3][:], src_qm[:, j + 3, :])

        if dbg is not None:
            pspc = sB.tile([32, 256], F32, tag="dbgpsp")
            nc.scalar.copy(pspc[:], psp[:])
            nc.sync.dma_start(dbg["d_psp"][:], pspc[:])
            nc.sync.dma_start(dbg["d_seq"][:], seq_buf[:])
            nc.sync.dma_start(dbg["d_ltgt"][:], ltgt_buf[:])
            nc.sync.dma_start(dbg["d_mx"][:], mx_buf[:])
            for k in range(NF):
                nc.sync.dma_start(
                    dbg["d_m8f"][k],
                    m8["f", k][:].rearrange("p c t -> p (c t)"))
                nc.sync.dma_start(
                    dbg["d_m8b"][k],
                    m8["b", k][:].rearrange("p c t -> p (c t)"))

        # ---------------- tails ----------------
        # q: loss + argmax-match count
        lnseq = sB.tile([128, L], F32, tag="lnseq")
        nc.scalar.activation(lnseq[:], seq_buf[:], AF.Ln)
        qcol = sB.tile([128, L], F32, tag="qcol")
        nc.vector.tensor_sub(qcol[:], lnseq[:], ltgt_buf[:])
        nc.vector.tensor_reduce(partq[:, 0:1], qcol[:], axis=AX.X, op=ALU.add)
        qeq = sB.tile([128, L], F32, tag="qeq")
        nc.vector.tensor_tensor(qeq[:], ltgt_buf[:], mx_buf[:], ALU.is_equal)
        nc.vector.tensor_reduce(partq[:, 2:3], qeq[:], axis=AX.X, op=ALU.add)

        # p: read collected logits [24, 256]
        gtb = sB.tile([24, 256], F32, tag="gtb")
        nc.vector.tensor_scalar(gtb[:], psp[0:24, :], 0.0, None, op0=ALU.is_gt)
        pdj = sB.tile([24, 256], F32, tag="pdj")
        nc.vector.scalar_tensor_tensor(pdj[:], gtb[:], 1.0, pw[:],
                                       op0=ALU.mult, op1=ALU.mult,
                                       accum_out=partp[:, 3:4])
        # softplus: fwd half signed by scp (-1 for target-1 rows), bwd half +1
        e1 = sB.tile([24, 128], F32, tag="e1")
        nc.scalar.activation(e1[:], psp[0:24, 0:128], AF.Exp, scale=scp[:, 0:1])
        l1 = sB.tile([24, 128], F32, tag="l1")
        nc.scalar.activation(l1[:], e1[:], AF.Ln, bias=ones32[0:24, 0:1])
        nc.vector.tensor_reduce(partp[:, 1:2], l1[:], axis=AX.X, op=ALU.add)
        # row 0 of the bwd half is exact zeros (start-zeroed, never written):
        # contributes 128*softplus(0) = 128*ln2, subtracted on the host.
        e0 = sB.tile([24, 128], F32, tag="e0")
        nc.scalar.activation(e0[:], psp[0:24, 128:256], AF.Exp)
        l0 = sB.tile([24, 128], F32, tag="l0")
        nc.scalar.activation(l0[:], e0[:], AF.Ln, bias=ones32[0:24, 0:1])
        ps0 = sB.tile([24, 1], F32, tag="ps0")
        nc.vector.tensor_reduce(ps0[:], l0[:], axis=AX.X, op=ALU.add)
        nc.vector.tensor_add(partp[:, 1:2], partp[:, 1:2], ps0[:])

        # final partition reduction: out[4,1] = partq.T@1 + partp.T@1
        ps_out = pB.tile([4, 1], F32, tag="out", bufs=1)
        nc.tensor.matmul(ps_out[:], partq[:, :], ones32[:, :],
                         start=True, stop=False)
        nc.tensor.matmul(ps_out[:], partp[:, :], ones32[0:24, :],
                         start=False, stop=True)
        out_sb = sB.tile([4, 1], F32, tag="outsb")
        nc.scalar.copy(out_sb[:], ps_out[:])
        nc.sync.dma_start(out_d[:], out_sb[:])


# ------------------------------------------------------------------
_PROGRAM = None

def _get_program():
    global _PROGRAM
    if _PROGRAM is None:
        _PROGRAM = build_program()
    return _PROGRAM


def _to_e4(x, scale):
    return np.clip(np.asarray(x, np.float32) * scale, -224.0, 224.0).astype(E4)


def _pack_w8(w):
    """[450, 450] fp32 -> [128, 4, 4, 128] fp8 x16 (fi,cin,cout,fo)"""
    out = np.zeros((128, 4, 4, 128), E4)
    wp = np.zeros((512, 512), np.float32)
    wp[:450, :450] = w
    v = wp.reshape(4, 128, 4, 128)          # [cin, fi, cout, fo]
    out[:] = _to_e4(np.transpose(v, (1, 0, 2, 3)), 16.0)
    return out


def _fm16(x, scale=1.0):
    """[trees(128), 450] fp32 -> [128, 4, 128] fp16 feature-major"""
    out = np.zeros((128, 4, 128), np.float16)
    xp = np.zeros((x.shape[0], 512), np.float32)
    xp[:, :450] = x * scale
    out[:] = np.transpose(xp.reshape(-1, 4, 128), (2, 1, 0))
    return out


def make_in_maps(wid, tree_vec, emb, W_w, W_b, U_w, U_b, Wo_w, Wo_b, Us_w,
                 Us_b, Wz_w, Wz_b, Wr_w, Ur_w, Ur_b, Wh_w, Wh_b):
    """Host-side shard + pack. Returns list of 8 per-core input dicts."""
    f32 = np.float32
    wid = np.asarray(wid); emb = np.asarray(emb, f32)
    tree_vec = np.asarray(tree_vec, f32)
    W_w, W_b = np.asarray(W_w, f32), np.asarray(W_b, f32)
    U_w, U_b = np.asarray(U_w, f32), np.asarray(U_b, f32)
    Wz_w, Wz_b = np.asarray(Wz_w, f32), np.asarray(Wz_b, f32)
    Wh_w, Wh_b = np.asarray(Wh_w, f32), np.asarray(Wh_b, f32)
    Wr_w = np.asarray(Wr_w, f32)
    Ur_w, Ur_b = np.asarray(Ur_w, f32), np.asarray(Ur_b, f32)
    Wo_w, Wo_b = np.asarray(Wo_w, f32), np.asarray(Wo_b, f32)
    Us_w, Us_b = np.asarray(Us_w, f32), np.asarray(Us_b, f32)
    assert float(Us_b[0]) == 0.0

    # vocab-sized precompute: [emb,1] @ [Wz1|Wh1|Wr|U1] + bias rows
    wpre = np.concatenate([Wz_w[:H], Wh_w[:H], Wr_w, U_w[:H]], 1)
    bias_row = np.concatenate([Wz_b, Wh_b, Ur_b, np.zeros(H, f32)])
    emb_pre = emb @ wpre + bias_row[None, :]           # [V, 1800]
    tvU = tree_vec @ U_w[2 * H:] + U_b                 # [T, H]
    tvW = tree_vec @ W_w[H:H + LAT] + W_b[None, :]     # [T, H]

    wo16 = np.zeros((128, 4, V), np.float16)
    wop = np.zeros((512, V), np.float32)
    wop[:450] = Wo_w
    wop[450] = Wo_b                                    # ones-row K slot
    wo16[:] = np.transpose(wop.reshape(4, 128, V), (1, 0, 2))

    us = Us_w[:, 0]
    usp = np.zeros(512, f32); usp[:450] = us
    ust = np.zeros((128, 4, L, 32), np.float16)
    for j in range(L):
        ust[:, :, j, j] = usp.reshape(4, 128).T

    identp = np.zeros((128, 3, 128), E4)
    identp[:, 1, :] = (np.eye(128) * 0.25).astype(E4)

    pwh = np.zeros((24, 256), np.float16)
    pwh[0, 0:128] = 1.0                                # root, target 1
    pwh[1:23, 0:128] = 1.0                             # fwd k=0..21, target 1
    pwh[23, 0:128] = -1.0                              # fwd k=22, target 0
    pwh[1:24, 128:256] = -1.0                          # bwd, target 0
    scph = np.full((24, 1), -1.0, f32)
    scph[23, 0] = 1.0

    shared = {
        "wz2": _pack_w8(Wz_w[H:]), "wh2": _pack_w8(Wh_w[H:]),
        "ur": _pack_w8(Ur_w), "u2": _pack_w8(U_w[H:2 * H]),
        "w1": _pack_w8(W_w[:H]),
        "wo": wo16, "ust": ust, "identp": identp,
        "ident16": np.eye(128, dtype=np.float16),
        "pw": pwh, "scp": scph,
        "ones32": np.ones((128, 1), f32),
    }

    in_maps = []
    for core in range(N_CORES):
        t0 = core * TC
        wc = wid[t0:t0 + TC]                           # [128, L]
        g = emb_pre[wc]                                # [128, L, 1800]
        gzhr = np.zeros((L, 128, 3, 4, 128), E4)
        gpn = np.zeros((L, 128, 4, 128), E4)
        for n in range(L):
            gn = g[:, n]                               # [tree, 1800]
            gpad = np.zeros((TC, 3, 512), np.float32)
            for kind in range(3):
                gpad[:, kind, :450] = gn[:, kind * H:(kind + 1) * H]
            # [tree, kind, c, p] -> [p, kind, c, tree]
            v = gpad.reshape(TC, 3, 4, 128)
            gzhr[POS[n]] = _to_e4(np.transpose(v, (3, 1, 2, 0)), 64.0)
            gpp = np.zeros((TC, 512), np.float32)
            gpp[:, :450] = gn[:, 3 * H:] + tvU[t0:t0 + TC]
            gpn[POS[n]] = _to_e4(
                np.transpose(gpp.reshape(TC, 4, 128), (2, 1, 0)), 64.0)

        qm = np.zeros((L, TC, V), np.float16)
        jj, tt = np.meshgrid(np.arange(L), np.arange(TC), indexing="ij")
        qm[jj, tt, wc.T] = 1.0

        m = dict(shared)
        m["gzhr"] = gzhr
        m["gp"] = gpn
        m["qmask"] = qm
        tvw16 = _fm16(tvW[t0:t0 + TC], 16.0)
        tvw16[66, 3, :] = 16.0      # -> relu(16/16)=1.0: Wo_b ones-row
        m["tvw"] = tvw16
        in_maps.append(m)
    return in_maps


def combine(outs):
    """outs: list of 8 [4,1] arrays -> reference 4-tuple."""
    s = np.sum([o[:, 0].astype(np.float64) for o in outs], axis=0)
    q_loss = np.float32(s[0] / T)
    p_loss = np.float32((s[1] - N_CORES * TC * np.log(2.0)) / T)
    q_acc = np.float32(np.float32(s[2]) / np.float32(L * T))
    p_cnt = s[3] + N_CORES * 24 * TC
    p_acc = np.float32(np.float32(p_cnt) / np.float32((NE + 1) * T))
    return (q_loss, p_loss, q_acc, p_acc)


def run_on_cores(in_maps, trace=False, **kw):
    nc = _get_program()
    return run_bass_kernel_spmd(nc, in_maps, list(range(N_CORES)),
                                trace=trace, **kw)


def kernel(**inputs):
    in_maps = make_in_maps(**inputs)
    res = run_on_cores(in_maps)
    return combine([res.results[c]["out"] for c in range(N_CORES)])
